# revision 7
# baseline (speedup 1.0000x reference)
"""Ternary (BitwiseLinear) matmul kernel for Trainium2, 8-core data-parallel.

y = ternary(x) @ ternary(w).T  with threshold 0.05, int-exact accumulation.

Sharding: x is split along the token dim across 8 cores (4096 tokens each);
the weight is replicated. Each core computes its y shard independently
(no collectives) and shards are concatenated on the host.

Per-core pipeline:
  1. quantize w and x tiles to ternary bf16: q = (v >= T) - (v <= -T)
     (two DVE tensor_scalar compares + one DVE tensor_tensor subtract;
     exact: no input element equals +-T bit-exactly, sums < 2^24)
  2. PE-transpose q tiles 128x128 into one bf16 PSUM bank, evict with a
     single copy per tile -> k-major layout for the matmul.
  3. bf16 matmuls (contraction over k on partitions) accumulate y tile
     [t:128, o:512] in PSUM f32, evict via ACT, DMA out.
"""

import contextlib
import threading

import numpy as np

N_CORES = 8
TOKENS = 32768
TOK_PER_CORE = TOKENS // N_CORES
K = 1024
O = 1024
P = 128
THR = 0.05

_cache = {}
_lock = threading.Lock()


def _split_multi_waits(nc):
    """walrus in this env can't encode >1 sync wait on one instruction: hoist
    extra waits into single-wait NOPs on the same engine, just before the
    instruction (identical per-engine wait semantics)."""
    import concourse.mybir as mybir

    uid = 0
    for f in nc.m.functions:
        for b in f.blocks:
            out = []
            changed = False
            for inst in b.instructions:
                si = inst.sync_info
                if si is not None and si.on_wait and len(si.on_wait) > 1:
                    waits = list(si.on_wait)
                    for w in waits[:-1]:
                        uid += 1
                        out.append(mybir.InstNoOp(
                            name=f"I-waitsplit-{uid}",
                            engine=inst.engine,
                            sync_info=mybir.SyncInfo(on_wait=[w], on_update=[]),
                        ))
                    inst.sync_info = mybir.SyncInfo(
                        on_wait=[waits[-1]], on_update=list(si.on_update))
                    changed = True
                out.append(inst)
            if changed:
                b.instructions = out


def build_nc(tokens=TOK_PER_CORE, loop_n=1):
    import concourse.bass as bass
    import concourse.mybir as mybir
    from concourse.masks import make_identity
    from concourse.tile import TileContext

    F32 = mybir.dt.float32
    BF16 = mybir.dt.bfloat16
    A = mybir.AluOpType

    KB = K // P          # 8 k-blocks
    n_ttiles = tokens // P

    nc = bass.Bass()
    x = nc.dram_tensor("x", [tokens, K], F32, kind="ExternalInput")
    w = nc.dram_tensor("weight", [O, K], F32, kind="ExternalInput")
    y = nc.dram_tensor("out", [tokens, O], F32, kind="ExternalOutput")

    with TileContext(nc) as tc:
        with (
            tc.tile_pool(name="const", bufs=1) as const_pool,
            tc.tile_pool(name="wqt", bufs=1) as wqt_pool,
            tc.tile_pool(name="xin", bufs=3) as xin_pool,
            tc.tile_pool(name="quant", bufs=3) as q_pool,
            tc.tile_pool(name="xqt", bufs=3) as xqt_pool,
            tc.tile_pool(name="yout", bufs=4) as y_pool,
            tc.tile_pool(name="psum_t", bufs=2, space="PSUM") as psumt_pool,
            tc.tile_pool(name="psum_y", bufs=4, space="PSUM") as psumy_pool,
        ):
            identity = const_pool.tile([P, P], BF16)
            make_identity(nc, identity)

            def quantize(src_tile):
                """f32 [128, K] -> ternary bf16 [128, K]: (v>=T) - (v<=-T)."""
                u = q_pool.tile([P, K], BF16, tag="q_u")
                nc.vector.tensor_scalar(
                    out=u[:], in0=src_tile[:], scalar1=THR, scalar2=None,
                    op0=A.is_ge)
                v = q_pool.tile([P, K], BF16, tag="q_v")
                nc.vector.tensor_scalar(
                    out=v[:], in0=src_tile[:], scalar1=-THR, scalar2=None,
                    op0=A.is_le)
                q = q_pool.tile([P, K], BF16, tag="q_q")
                nc.vector.tensor_tensor(out=q[:], in0=u[:], in1=v[:],
                                        op=A.subtract)
                return q

            def transpose_to(q, dst, evict_engine):
                """q bf16 [128, K] natural -> dst bf16 [128, KB, 128] k-major."""
                ps = psumt_pool.tile([P, KB, P], BF16, tag="psT")
                for kb in range(KB):
                    nc.tensor.transpose(
                        ps[:, kb, :], q[:, kb * P:(kb + 1) * P], identity)
                if evict_engine == "vector":
                    nc.vector.tensor_copy(dst[:], ps[:])
                else:
                    nc.scalar.copy(dst[:], ps[:])

            # --- weight phase: wqT [k_part, k_blk, o] ---
            wqT = wqt_pool.tile([P, KB, O], BF16)
            for ob in range(O // P):
                wt = xin_pool.tile([P, K], F32, tag="w_in")
                nc.sync.dma_start(wt[:], w[ob * P:(ob + 1) * P, :])
                qw = quantize(wt)
                transpose_to(qw, wqT[:, :, ob * P:(ob + 1) * P],
                             "vector" if ob % 2 else "scalar")

            def main_body():
                for tb in range(n_ttiles):
                    xt = xin_pool.tile([P, K], F32, tag="x_in")
                    nc.sync.dma_start(xt[:], x[tb * P:(tb + 1) * P, :])
                    qx = quantize(xt)
                    xqT = xqt_pool.tile([P, KB, P], BF16, tag="xqT")
                    transpose_to(qx, xqT, "vector" if tb % 2 else "scalar")

                    for oh in range(2):
                        yp = psumy_pool.tile([P, 512], F32, tag="yp")
                        for kb in range(KB):
                            nc.tensor.matmul(
                                yp[:],
                                xqT[:, kb, :],
                                wqT[:, kb, oh * 512:(oh + 1) * 512],
                                start=(kb == 0),
                                stop=(kb == KB - 1),
                            )
                        ysb = y_pool.tile([P, 512], F32, tag="ysb")
                        nc.scalar.copy(ysb[:], yp[:])
                        nc.sync.dma_start(
                            y[tb * P:(tb + 1) * P, oh * 512:(oh + 1) * 512],
                            ysb[:])

            # loop_n > 1 wraps the token loop in a hardware loop purely for
            # benchmarking (amortizes per-call host/PJRT overhead).
            if loop_n > 1:
                with tc.For_i(0, loop_n, 1):
                    main_body()
            else:
                main_body()

    _split_multi_waits(nc)
    return nc


def _get_nc(tokens=TOK_PER_CORE):
    with _lock:
        if tokens not in _cache:
            _cache[tokens] = build_nc(tokens)
        return _cache[tokens]


def kernel(x: np.ndarray, weight: np.ndarray):
    from concourse.bass_utils import run_bass_kernel_spmd

    x = np.ascontiguousarray(x, dtype=np.float32)
    weight = np.ascontiguousarray(weight, dtype=np.float32)
    assert x.shape == (TOKENS, K) and weight.shape == (O, K)

    nc = _get_nc()
    in_maps = [
        {"x": x[i * TOK_PER_CORE:(i + 1) * TOK_PER_CORE], "weight": weight}
        for i in range(N_CORES)
    ]
    res = run_bass_kernel_spmd(nc, in_maps, core_ids=list(range(N_CORES)))
    return np.concatenate([r["out"] for r in res.results], axis=0)
